# revision 11
# baseline (speedup 1.0000x reference)
"""VQ codebook assignment kernel for Trainium2 (8 NeuronCores, data-parallel).

Problem: x (8, 4096, 512) f32, codebook (8192, 512) f32
         -> codes (8, 4096) int32 = argmin_k ||x - c_k||^2

Strategy:
  - Shard rows (B*N = 32768) across 8 cores, 4096 rows each; replicate codebook.
  - argmin_k(||c||^2 - 2 x.c)  ==  argmax_k(2 x.c - ||c||^2)  (drop ||x||^2).
  - Native fp32 matmuls (4 cyc/row on PE): measured max arithmetic error
    ~1.7e-5, safely below the smallest top-2 distance gap in the data (8.9e-5).
    Cheaper schemes (bf16 hi/lo 3/4-pass, tf32-style splits) measure 1e-4..1e-3
    error and flip argmins on near-tie rows.
  - The -||c||^2 bias is folded into the PSUM accumulation as an extra K=4
    bf16 matmul (ones(4,128).T @ bias4), bias4 = 4-way bf16 split of -||c||^2
    (error ~1e-7).
  - Per 128-row tile: accumulate scores for all 8192 codes into an fp32 SBUF
    strip (128 x 8192) via ScalarE PSUM->SBUF copies, then one
    vector.max + vector.max_index scan gives the argmax index per row.
"""

import numpy as np
import ml_dtypes

import concourse.mybir as mybir
from concourse import bacc
from concourse.tile import TileContext
from concourse import bass_utils

TILE_P = 128  # rows per tile (PSUM partitions)
CODE_BLK = 512  # codes per PSUM bank (fp32)
NCORES = 8


def build_nc(n_tiles: int, d_chunks: int, n_codes: int, repeat: int = 1):
    dt = mybir.dt
    n_blks = n_codes // CODE_BLK
    nc = bacc.Bacc("TRN2")

    x = nc.dram_tensor("x", (n_tiles, TILE_P, d_chunks, TILE_P), dt.float32,
                       kind="ExternalInput")
    cb = nc.dram_tensor("cb", (TILE_P, d_chunks, n_codes), dt.float32,
                        kind="ExternalInput")
    bias = nc.dram_tensor("bias", (4, n_codes), dt.bfloat16,
                          kind="ExternalInput")
    out = nc.dram_tensor("codes", (n_tiles, TILE_P, 1), dt.uint32,
                         kind="ExternalOutput")

    with TileContext(nc) as tc:
        with (
            tc.tile_pool(name="cb", bufs=1) as cb_pool,
            tc.tile_pool(name="xt", bufs=3) as x_pool,
            tc.tile_pool(name="score", bufs=1) as score_pool,
            tc.tile_pool(name="small", bufs=2) as small_pool,
            tc.tile_pool(name="psum", bufs=4, space="PSUM") as psum_pool,
        ):
            cb_sb = cb_pool.tile([TILE_P, d_chunks, n_codes], dt.float32, tag="cb")
            bias_sb = cb_pool.tile([4, n_codes], dt.bfloat16, tag="bias")
            ones_sb = cb_pool.tile([4, TILE_P], dt.bfloat16, tag="ones")

            nc.vector.memset(ones_sb, 1.0)
            nc.sync.dma_start(bias_sb, bias[:, :])
            # load codebook block-by-block so the first matmuls can start early
            for b in range(n_blks):
                s = slice(b * CODE_BLK, (b + 1) * CODE_BLK)
                nc.sync.dma_start(cb_sb[:, :, s], cb[:, :, s])

            def tile_body(t):
                x_sb = x_pool.tile([TILE_P, d_chunks, TILE_P], dt.float32, tag="x")
                nc.sync.dma_start(x_sb, x[t])

                score_sb = score_pool.tile([TILE_P, n_codes], dt.float32,
                                           tag="score")
                for b in range(n_blks):
                    s = slice(b * CODE_BLK, (b + 1) * CODE_BLK)
                    ps = psum_pool.tile([TILE_P, CODE_BLK], dt.float32, tag="ps")
                    # bias: ones(4,128).T @ bias4(4,512) broadcasts -||c||^2
                    nc.tensor.matmul(ps, lhsT=ones_sb, rhs=bias_sb[:, s],
                                     start=True, stop=False)
                    for c in range(d_chunks):
                        nc.tensor.matmul(ps, lhsT=x_sb[:, c, :],
                                         rhs=cb_sb[:, c, s],
                                         start=False, stop=(c == d_chunks - 1))
                    nc.scalar.copy(score_sb[:, s], ps)

                max8 = small_pool.tile([TILE_P, 8], dt.float32, tag="max8")
                idx8 = small_pool.tile([TILE_P, 8], dt.uint32, tag="idx8")
                nc.vector.max(out=max8, in_=score_sb)
                nc.vector.max_index(out=idx8, in_max=max8, in_values=score_sb)
                nc.sync.dma_start(out[t], idx8[:, 0:1])

            def all_tiles():
                for t in range(n_tiles):
                    tile_body(t)

            if repeat == 1:
                all_tiles()
            else:
                with tc.For_i(0, repeat, 1):
                    all_tiles()

    nc.compile()
    return nc


def _prep_x_core(x_core: np.ndarray, d_chunks: int):
    """x_core: (rows, D) fp32 -> (t, p, c, m) with d = c*128+p, row = t*128+m."""
    rows, D = x_core.shape
    n_tiles = rows // TILE_P
    xt = np.ascontiguousarray(x_core.T)  # (D, rows)
    a = xt.reshape(d_chunks, TILE_P, n_tiles, TILE_P)  # (c, p, t, m)
    return np.ascontiguousarray(a.transpose(2, 1, 0, 3))


def prep_inputs(x_flat: np.ndarray, codebook: np.ndarray):
    R, D = x_flat.shape
    K = codebook.shape[0]
    d_chunks = D // TILE_P
    rows_per_core = R // NCORES
    n_tiles = rows_per_core // TILE_P

    # codebook device layout: cb[p, c, n] = 2*codebook[n, c*128+p]
    c2T = np.ascontiguousarray((2.0 * codebook).T.astype(np.float32))  # (D, K)
    cb_lay = np.ascontiguousarray(
        c2T.reshape(d_chunks, TILE_P, K).transpose(1, 0, 2))

    # 4-way bf16 split of -||c||^2 (fp64-exact norms)
    r = (-(codebook.astype(np.float64) ** 2).sum(axis=1)).astype(np.float32)
    parts = []
    for _ in range(4):
        p = r.astype(ml_dtypes.bfloat16)
        parts.append(p)
        r = r - p.astype(np.float32)
    bias4 = np.ascontiguousarray(np.stack(parts, axis=0))  # (4, K)

    in_maps = []
    for i in range(NCORES):
        xc = x_flat[i * rows_per_core:(i + 1) * rows_per_core]
        in_maps.append({
            "x": _prep_x_core(xc, d_chunks),
            "cb": cb_lay,
            "bias": bias4,
        })
    return in_maps, n_tiles, d_chunks


def run(x_flat: np.ndarray, codebook: np.ndarray, repeat: int = 1):
    R, D = x_flat.shape
    K = codebook.shape[0]
    in_maps, n_tiles, d_chunks = prep_inputs(x_flat, codebook)
    nc = build_nc(n_tiles, d_chunks, K, repeat=repeat)
    res = bass_utils.run_bass_kernel_spmd(
        nc, in_maps, core_ids=list(range(NCORES)))
    codes = np.concatenate(
        [res.results[i]["codes"].reshape(-1).astype(np.int32)
         for i in range(NCORES)])
    return codes, res


def kernel(x: np.ndarray, codebook: np.ndarray) -> np.ndarray:
    x = np.ascontiguousarray(np.asarray(x), dtype=np.float32)
    codebook = np.ascontiguousarray(np.asarray(codebook), dtype=np.float32)
    B, N, D = x.shape
    codes, _ = run(x.reshape(-1, D), codebook)
    return codes.reshape(B, N).astype(np.int32)


# revision 13
# speedup vs baseline: 1.3310x; 1.3310x over previous
"""VQ codebook assignment kernel for Trainium2 (8 NeuronCores, data-parallel).

Problem: x (8, 4096, 512) f32, codebook (8192, 512) f32
         -> codes (8, 4096) int32 = argmin_k ||x - c_k||^2

Strategy:
  - Shard rows (B*N = 32768) across 8 cores, 4096 rows each; replicate codebook.
  - argmin_k ||x-c_k||^2 == argmax_k( x.(2c_k) - (||c_k||^2 - 512) )
    (drop ||x||^2; the 512 centering shrinks fp32 ulp in the accumulator).
  - Matmul: fp16 hi/lo split of both operands, 3 passes
    (xh.ch + xh.cl + xl.ch) at full PE rate (1 cyc/row; fp16 denormals are
    preserved by the PE — verified on HW). Split residuals are ~2^-22, so the
    total score error is dominated by fp32 PSUM accumulation (~1e-5), safely
    below the smallest top-2 distance gap in the data (8.9e-5). Verified
    exact vs the fp32 reference on the real (seeded) data.
    (bf16 3-pass errs ~1e-3 and flips argmins; native fp32 matmul is exact
    but 4x slower per row.)
  - The bias (512 - ||c||^2, exact fp32) is added during the PSUM->SBUF copy
    as a DVE tensor_tensor add against a partition-replicated bias strip.
  - Per 128-row tile: accumulate scores for all 8192 codes into an fp32 SBUF
    strip (128 x 8192), then one vector.max + vector.max_index scan gives the
    argmax index per row.
"""

import numpy as np

import concourse.mybir as mybir
from concourse import bacc
from concourse.tile import TileContext
from concourse import bass_utils

TILE_P = 128  # rows per tile (PSUM partitions)
CODE_BLK = 512  # codes per PSUM bank (fp32)
NCORES = 8
CENTER = 512.0  # subtracted from ||c||^2 to center scores near 0


def build_nc(n_tiles: int, d_chunks: int, n_codes: int, repeat: int = 1):
    dt = mybir.dt
    n_blks = n_codes // CODE_BLK
    nc = bacc.Bacc("TRN2")

    xh = nc.dram_tensor("xh", (n_tiles, TILE_P, d_chunks, TILE_P), dt.float16,
                        kind="ExternalInput")
    xl = nc.dram_tensor("xl", (n_tiles, TILE_P, d_chunks, TILE_P), dt.float16,
                        kind="ExternalInput")
    cbh = nc.dram_tensor("cbh", (TILE_P, d_chunks, n_codes), dt.float16,
                         kind="ExternalInput")
    cbl = nc.dram_tensor("cbl", (TILE_P, d_chunks, n_codes), dt.float16,
                         kind="ExternalInput")
    bias = nc.dram_tensor("bias", (TILE_P, n_codes), dt.float32,
                          kind="ExternalInput")
    out = nc.dram_tensor("codes", (n_tiles, TILE_P, 1), dt.uint32,
                         kind="ExternalOutput")

    with TileContext(nc) as tc:
        with (
            tc.tile_pool(name="cb", bufs=1) as cb_pool,
            tc.tile_pool(name="xt", bufs=3) as x_pool,
            tc.tile_pool(name="score", bufs=1) as score_pool,
            tc.tile_pool(name="small", bufs=2) as small_pool,
            tc.tile_pool(name="psum", bufs=8, space="PSUM") as psum_pool,
        ):
            cbh_sb = cb_pool.tile([TILE_P, d_chunks, n_codes], dt.float16, tag="cbh")
            cbl_sb = cb_pool.tile([TILE_P, d_chunks, n_codes], dt.float16, tag="cbl")
            bias_sb = cb_pool.tile([TILE_P, n_codes], dt.float32, tag="bias")

            nc.sync.dma_start(bias_sb, bias[:, :])
            # load codebook block-by-block so the first matmuls can start early
            for b in range(n_blks):
                s = slice(b * CODE_BLK, (b + 1) * CODE_BLK)
                nc.sync.dma_start(cbh_sb[:, :, s], cbh[:, :, s])
                nc.sync.dma_start(cbl_sb[:, :, s], cbl[:, :, s])

            def tile_body(t):
                xh_sb = x_pool.tile([TILE_P, d_chunks, TILE_P], dt.float16, tag="xh")
                xl_sb = x_pool.tile([TILE_P, d_chunks, TILE_P], dt.float16, tag="xl")
                nc.sync.dma_start(xh_sb, xh[t])
                nc.sync.dma_start(xl_sb, xl[t])

                score_sb = score_pool.tile([TILE_P, n_codes], dt.float32,
                                           tag="score")
                for b in range(n_blks):
                    s = slice(b * CODE_BLK, (b + 1) * CODE_BLK)
                    ps = psum_pool.tile([TILE_P, CODE_BLK], dt.float32, tag="ps")
                    k = 0
                    # passes: xh.ch, xh.cl, xl.ch
                    for xs, cs in ((xh_sb, cbh_sb), (xh_sb, cbl_sb),
                                   (xl_sb, cbh_sb)):
                        for c in range(d_chunks):
                            nc.tensor.matmul(ps, lhsT=xs[:, c, :],
                                             rhs=cs[:, c, s],
                                             start=(k == 0),
                                             stop=(k == 3 * d_chunks - 1))
                            k += 1
                    # PSUM -> SBUF with exact bias add (DVE)
                    nc.vector.tensor_add(score_sb[:, s], ps, bias_sb[:, s])

                max8 = small_pool.tile([TILE_P, 8], dt.float32, tag="max8")
                idx8 = small_pool.tile([TILE_P, 8], dt.uint32, tag="idx8")
                nc.vector.max(out=max8, in_=score_sb)
                nc.vector.max_index(out=idx8, in_max=max8, in_values=score_sb)
                nc.sync.dma_start(out[t], idx8[:, 0:1])

            def all_tiles():
                for t in range(n_tiles):
                    tile_body(t)

            if repeat == 1:
                all_tiles()
            else:
                with tc.For_i(0, repeat, 1):
                    all_tiles()

    nc.compile()
    return nc


def _f16_split(a32: np.ndarray):
    hi = a32.astype(np.float16)
    lo = (a32 - hi.astype(np.float32)).astype(np.float16)
    return hi, lo


def _lay_x(a: np.ndarray, d_chunks: int, n_tiles: int):
    """(D, rows) -> (t, p, c, m) with d = c*128+p, row = t*128+m."""
    a = a.reshape(d_chunks, TILE_P, n_tiles, TILE_P)
    return np.ascontiguousarray(a.transpose(2, 1, 0, 3))


def prep_inputs(x_flat: np.ndarray, codebook: np.ndarray):
    R, D = x_flat.shape
    K = codebook.shape[0]
    d_chunks = D // TILE_P
    rows_per_core = R // NCORES
    n_tiles = rows_per_core // TILE_P

    # codebook device layout: [p, c, n] = 2*codebook[n, c*128+p], fp16 hi/lo
    c2T = np.ascontiguousarray((2.0 * codebook).T.astype(np.float32))  # (D, K)
    ch, cl = _f16_split(c2T)

    def lay_cb(a):
        return np.ascontiguousarray(
            a.reshape(d_chunks, TILE_P, K).transpose(1, 0, 2))

    cbh_lay, cbl_lay = lay_cb(ch), lay_cb(cl)

    # exact fp32 bias strip, replicated over partitions: 512 - ||c||^2
    brow = (CENTER - (codebook.astype(np.float64) ** 2).sum(axis=1)).astype(
        np.float32)
    bias_rep = np.ascontiguousarray(np.broadcast_to(brow[None, :], (TILE_P, K)))

    in_maps = []
    for i in range(NCORES):
        xc = x_flat[i * rows_per_core:(i + 1) * rows_per_core]
        xt = np.ascontiguousarray(xc.T).astype(np.float32)  # (D, rows)
        xh, xl = _f16_split(xt)
        in_maps.append({
            "xh": _lay_x(xh, d_chunks, n_tiles),
            "xl": _lay_x(xl, d_chunks, n_tiles),
            "cbh": cbh_lay,
            "cbl": cbl_lay,
            "bias": bias_rep,
        })
    return in_maps, n_tiles, d_chunks


def run(x_flat: np.ndarray, codebook: np.ndarray, repeat: int = 1):
    K = codebook.shape[0]
    in_maps, n_tiles, d_chunks = prep_inputs(x_flat, codebook)
    nc = build_nc(n_tiles, d_chunks, K, repeat=repeat)
    res = bass_utils.run_bass_kernel_spmd(
        nc, in_maps, core_ids=list(range(NCORES)))
    codes = np.concatenate(
        [res.results[i]["codes"].reshape(-1).astype(np.int32)
         for i in range(NCORES)])
    return codes, res


def kernel(x: np.ndarray, codebook: np.ndarray) -> np.ndarray:
    x = np.ascontiguousarray(np.asarray(x), dtype=np.float32)
    codebook = np.ascontiguousarray(np.asarray(codebook), dtype=np.float32)
    B, N, D = x.shape
    codes, _ = run(x.reshape(-1, D), codebook)
    return codes.reshape(B, N).astype(np.int32)
